# revision 127
# baseline (speedup 1.0000x reference)
"""DCT Frequency Splitter — Trainium2 Bass kernel (v3, bf16 end-to-end).

Math: FFT2 -> mask -> IFFT2 -> real is a linear operator on the 196 patch
tokens (per channel): z = A @ patches with A = Re(Finv diag(m) F) (196x196,
real, built on host from the 4 mask params).  With shared mask params the
high path is high = patches - z, so one matmul feeds both outputs:
lo = s_l * z, hi = s_h * (patches - z).

v3 layout decisions (all driven by the TimelineSim cost model):
- bf16 everywhere off-chip: x is converted to bf16 on the host, outputs are
  stored bf16 and upcast on the host.  Halves DMA traffic (the kernel is
  DMA-bound at ~360 GB/s/core); rel-err budget 2e-2 >> bf16's ~4e-3.
- gate-first: per-image token means are computed straight from the x tiles
  with tiny K-contraction matmuls into a PSUM arena (gT in [d, img] layout,
  no PSUM row drain + transpose shuffle), so the gate scales are ready when
  the main matmuls drain and the lo output leaves PSUM already scaled - one
  Activation pass instead of two.
- hi is reconstructed OFF PSUM from the already-drained lo tile:
  hi = crh*x - (crh/crl)*lo via all-SBUF bf16 muls/adds (DVE 2x/4x modes),
  so PSUM has a single reader (the Act drains) and frees fast.
- engine split per image: Act = scaled lo drains (+ sigmoids), DVE = hi
  reconstruction, Pool/GPSIMD = b-side hi scale (it may not touch PSUM),
  PE = matmuls.
- group-batched load DMAs and per-2-image output tiles/stores: dependency
  tracking is tile-granular, so a store's source tile must not outlive its
  own 2 images, or it waits on the whole group.

Sharding: pure data parallel, batch 128 -> 16 per core across 8 cores.
"""

import os
import numpy as np

import concourse.bass as bass
import concourse.bacc as bacc_mod
import concourse.mybir as mybir
import concourse.tile as tile
from concourse.bass_utils import run_bass_kernel_spmd
from concourse.tile_rust import add_dep_helper

H, W = 14, 14
B, N, D = 128, 197, 768
P = 196  # patch tokens
NCORES = 8
BS = B // NCORES  # batches per core

GRP = int(os.environ.get("KRN_GROUP", "4"))        # images per group
BUFO = int(os.environ.get("KRN_BUFO", "5"))        # output half-tile buffers
F32 = mybir.dt.float32
BF16 = mybir.dt.bfloat16

# x tiles hold tokens 1..196 only (CLS skipped at load): xga = patches
# 0..127 on partitions 0..127, xgb = patches 128..195 on partitions 0..67,
# so matmul operands/outputs and the elementwise hi ops all share base
# partition 0 (the PE requires operand base partition in {0, 32, 64}).
# wtblob column layout (bf16, [128, 400])
WTA0 = 0      # A^T rows 0:128 (K = patches 0..127), cols 0:196
WTB0 = 196    # A^T rows 128:196 (K = patches 128..195), cols 196:392
ONES0 = 392   # [128, 1] column of 1/196
# gate consts: gblob [128,1152] = w1 only; w2blob [96,4]; rowblob [1,592]
# (b1 | ones16 | alr | ahr | cneg) - splitting off the partition-0-only
# rows avoids DMAing a mostly-zero [128 x 596] rectangle (~0.34us)
RB_B1, RB_ONES, RB_ALR, RB_AHR, RB_CNEG = 0, 192, 208, 336, 464


def _freq_mask_np(params, low):
    ch, cw, radius, sharp = [np.float64(v) for v in np.asarray(params)]
    y = np.arange(H, dtype=np.float64)
    x = np.arange(W, dtype=np.float64)
    d2 = (y[:, None] - ch) ** 2 + (x[None, :] - cw) ** 2
    dist = np.sqrt(d2 + 1e-12)
    s = np.clip(sharp, 0.5, 10.0)
    r = np.clip(radius, 1.0, min(H, W) / 2.0)
    m = np.exp(-((dist / r) ** s))
    return m if low else 1.0 - m


def _conv_operator(mask):
    """Real 196x196 operator equivalent to ifft2(fft2(img)*mask).real."""
    F_H = np.exp(-2j * np.pi * np.outer(np.arange(H), np.arange(H)) / H)
    F_W = np.exp(-2j * np.pi * np.outer(np.arange(W), np.arange(W)) / W)
    Fi_H = np.conj(F_H) / H
    Fi_W = np.conj(F_W) / W
    op = np.kron(Fi_H, Fi_W) @ np.diag(mask.ravel()) @ np.kron(F_H, F_W)
    return np.real(op)


def _build_program(consts, b2lo, b2hi):
    nc = bacc_mod.Bacc(None)

    xs_h = nc.dram_tensor("xs", [BS, N, D], BF16, kind="ExternalInput")
    lo_h = nc.dram_tensor("lo", [BS, N, D], BF16, kind="ExternalOutput")
    hi_h = nc.dram_tensor("hi", [BS, N, D], BF16, kind="ExternalOutput")

    ch = {k: nc.inline_tensor(v, name=f"c_{k}") for k, v in consts.items()}

    Copy = mybir.ActivationFunctionType.Copy
    Sig = mybir.ActivationFunctionType.Sigmoid

    # two small leading groups ramp the store stream earliest (measured
    # best); KRN_GPAT overrides for experiments
    if os.environ.get("KRN_GPAT"):
        sizes = [int(v) for v in os.environ["KRN_GPAT"].split(",")]
        assert sum(sizes) == BS
    elif BS == 16 and GRP == 4:
        sizes = [2, 2, 4, 4, 4]
    else:
        sizes = [min(GRP, BS - s) for s in range(0, BS, GRP)]
    groups, s = [], 0
    for sz in sizes:
        groups.append(list(range(s, s + sz)))
        s += sz
    n_groups = len(groups)

    with tile.TileContext(nc) as tc:
        with (
            tc.tile_pool(name="consts", bufs=1) as cp,
            tc.tile_pool(name="xp", bufs=n_groups) as xp,
            tc.tile_pool(name="outp", bufs=BUFO) as outp,
            tc.tile_pool(name="scr", bufs=4) as scr,
            tc.tile_pool(name="gp", bufs=2) as gp,
            tc.tile_pool(name="pm", bufs=2, space="PSUM") as pm,
            tc.tile_pool(name="par", bufs=2, space="PSUM") as par,
        ):
            def cload(key, dtype):
                arr = consts[key]
                t = cp.tile(list(arr.shape), dtype, tag=key)
                nc.sync.dma_start(out=t[:], in_=ch[key][...])
                return t

            wb = cload("wtblob", BF16)      # [128, 400]
            onescol = wb[:, ONES0:ONES0 + 1]

            def load_group(g):
                bs = groups[g]
                b0, gn = bs[0], len(bs)
                xga = xp.tile([128, GRP, D], BF16, tag="xga")
                xgb = xp.tile([68, GRP, D], BF16, tag="xgb")
                nc.sync.dma_start(
                    out=xga[:, 0:gn, :],
                    in_=xs_h[b0:b0 + gn, 1:129, :].rearrange("b t d -> t b d"))
                nc.sync.dma_start(
                    out=xgb[:, 0:gn, :],
                    in_=xs_h[b0:b0 + gn, 129:197, :].rearrange("b t d -> t b d"))
                return xga, xgb

            # group 0's loads go before the remaining consts so its gate
            # chain starts as early as possible; all loads precede all
            # stores so SP's in-order sequencer never parks a semaphore-
            # blocked store ahead of a ready load
            xt = {0: load_group(0)}

            gb = cload("gblob", BF16)       # [128, 1152] (w1)
            w2b = cload("w2blob", BF16)     # [96, 4]
            rb = cload("rowblob", BF16)     # [1, 592]
            w1c = gb[:, :].rearrange("p (a h b) -> p a h b", a=6, h=2)
            w2c0 = w2b[0:96, 0:2]
            w2c1 = w2b[0:96, 2:4]
            ones1 = rb[0:1, RB_ONES:RB_ONES + 16]
            alr = rb[0:1, RB_ALR:RB_ALR + 128]
            ahr = rb[0:1, RB_AHR:RB_AHR + 128]
            cneg = rb[0:1, RB_CNEG:RB_CNEG + 128]

            # dummy activation so the act-func table load (~1.3us) runs at
            # t~=1us instead of stalling the first group's gate sigmoid;
            # reads the earliest const so it never blocks Act's queue
            warm = gp.tile([1, 16], F32, tag="warm")
            nc.scalar.activation(warm[:], wb[0:1, 0:16], Sig)

            for g in range(1, n_groups):
                xt[g] = load_group(g)

            # CLS passthrough for all batches (DRAM -> DRAM), issued from
            # the near-free GPSIMD DMA queue so SP only handles bulk I/O;
            # emitted after the loads so its transfers never wedge between
            # the weight blob and the first x tile on the DMA FIFO
            nc.gpsimd.dma_start(out=lo_h[:, 0:1, :], in_=xs_h[:, 0:1, :])
            nc.gpsimd.dma_start(out=hi_h[:, 0:1, :], in_=xs_h[:, 0:1, :])

            def gate_chain(g, xga, xgb):
                bs = groups[g]
                gn = len(bs)
                # gate for this group, ahead of the main matmuls.
                # gT[d, j] = mean over patch tokens of x (tiny K-contraction
                # matmuls straight into the psum arena, [d, img] layout).
                arena = par.tile([128, 384], F32, tag="arena")
                for j in range(gn):
                    for c in range(6):
                        col = c * 16 + j
                        nc.tensor.matmul(
                            arena[:, col:col + 1],
                            xga[:, j, c * 128:(c + 1) * 128],
                            onescol[:],
                            start=True, stop=False)
                        nc.tensor.matmul(
                            arena[:, col:col + 1],
                            xgb[0:68, j, c * 128:(c + 1) * 128],
                            onescol[0:68],
                            start=False, stop=True)
                gTt = gp.tile([128, 6, 16], BF16, tag="gTt")
                nc.vector.tensor_copy(
                    gTt[:].rearrange("p a b -> p (a b)"), arena[:, 0:96])

                # hidden layer directly in transposed [feature, img] layout
                # (w1 chunks stationary): no transposes, tiny moving dims
                gate_pe = None
                for h in range(2):
                    hps = arena[0:96, 96 + 16 * h:96 + 16 * h + 16]
                    for c in range(6):
                        nc.tensor.matmul(hps[:, 0:gn], w1c[:, c, h, :],
                                         gTt[:, c, 0:gn],
                                         start=(c == 0), stop=False)
                    gate_pe = nc.tensor.matmul(
                        hps[:, 0:gn], rb[0:1, 96 * h:96 * h + 96],
                        ones1[0:1, 0:gn], start=False, stop=True)
                hTt = gp.tile([96, 2, 16], BF16, tag="hTt")
                for h in range(2):
                    hps = arena[0:96, 96 + 16 * h:96 + 16 * h + 16]
                    nc.vector.tensor_relu(hTt[:, h, 0:gn], hps[:, 0:gn])

                crows = []
                for col, b2f in ((0, b2lo), (1, b2hi)):
                    g_ps = arena[0:1, 128 + 16 * col:144 + 16 * col]
                    nc.tensor.matmul(g_ps[:, 0:gn], w2c0[:, col:col + 1],
                                     hTt[:, 0, 0:gn], start=True, stop=False)
                    nc.tensor.matmul(g_ps[:, 0:gn], w2c1[:, col:col + 1],
                                     hTt[:, 1, 0:gn], start=False, stop=True)
                    cr = gp.tile([1, 16], BF16, tag=f"crow{col}")
                    nc.scalar.activation(cr[:, 0:gn], g_ps[:, 0:gn], Sig,
                                         bias=b2f)
                    crows.append(cr)
                # per-image hi/lo gate ratio (the hi path is reconstructed
                # from the already-scaled lo tile: hi = crh*x - r*lo with
                # r = crh/crl; the -alpha ratio constant lives in cneg)
                rcp = gp.tile([1, 16], F32, tag="rcp")
                nc.vector.reciprocal(rcp[:, 0:gn], crows[0][0:1, 0:gn])
                rrow = gp.tile([1, 16], BF16, tag="rrow")
                nc.vector.tensor_mul(rrow[:, 0:gn], rcp[:, 0:gn],
                                     crows[1][0:1, 0:gn])
                # replicate the gate rows across partitions; the alpha
                # sigmoid is folded into the alr/ahr/cneg weight rows
                for k, (wrow, mov) in enumerate(
                        ((alr, crows[0][0:1, 0:gn]),
                         (ahr, crows[1][0:1, 0:gn]),
                         (cneg, rrow[0:1, 0:gn]))):
                    nc.tensor.matmul(arena[:, 160 + 16 * k:160 + 16 * k + gn],
                                     wrow[0:1, :], mov, start=True, stop=True)
                crlh = gp.tile([128, 48], F32, tag="crlh")
                nc.vector.tensor_copy(crlh[:], arena[:, 160:208])
                return crlh, gate_pe

            # gates run two groups ahead of their bodies so their small
            # DVE/Act steps never queue behind a full body's engine work
            gates = {g: gate_chain(g, *xt[g]) for g in range(min(2, n_groups))}

            for g, bs in enumerate(groups):
                gn = len(bs)
                b0 = bs[0]
                xga, xgb = xt[g]
                crlh, gate_pe = gates.pop(g)
                crl = crlh[:, 0:16]
                crh = crlh[:, 16:32]
                rneg = crlh[:, 32:48]

                # ---- main matmuls + scaled drains per image
                for j in range(gn):
                    # per-2-image output tiles: a store only waits on its
                    # own half's drains/adds (tile-granular dependency
                    # tracking would otherwise park it behind the whole
                    # group), and dedicated hi tiles release the x tiles to
                    # the pool at the final add rather than at the store
                    jj = j % 2
                    if jj == 0:
                        lo_ga = outp.tile([128, 2, D], BF16, tag="lo_ga")
                        lo_gb = outp.tile([68, 2, D], BF16, tag="lo_gb")
                        ho_ga = outp.tile([128, 2, D], BF16, tag="ho_ga")
                        ho_gb = outp.tile([68, 2, D], BF16, tag="ho_gb")
                    # PSUM packing: 3 banks/image — za_t[128,1024] holds
                    # za(0:768) + zb's tail chunk (768:1024), zb1 the rest.
                    # The a-side then drains in ONE 768-col Act pass; the
                    # two tiles free independently (a-side earlier), which
                    # beats a fully-merged 3-bank tile.
                    za_t = pm.tile([128, 1024], F32, tag="za_t")
                    zb1 = pm.tile([68, 512], F32, tag="zb1")
                    za_ch = [(0, 512, za_t[:, 0:512]),
                             (512, 768, za_t[:, 512:768])]
                    zb_ch = [(0, 512, zb1[0:68, :]),
                             (512, 768, za_t[0:68, 768:1024])]
                    for (n0, n1, zc) in za_ch:
                        mm = nc.tensor.matmul(zc, wb[:, 0:128],
                                              xga[:, j, n0:n1],
                                              start=True, stop=False)
                        add_dep_helper(mm.ins, gate_pe.ins,
                                       reason="gate chain schedules first")
                        nc.tensor.matmul(zc, wb[0:68, WTB0:WTB0 + 128],
                                         xgb[0:68, j, n0:n1],
                                         start=False, stop=True)
                    for (n0, n1, zc) in zb_ch:
                        mm = nc.tensor.matmul(zc, wb[:, 128:196],
                                              xga[:, j, n0:n1],
                                              start=True, stop=False)
                        add_dep_helper(mm.ins, gate_pe.ins,
                                       reason="gate chain schedules first")
                        nc.tensor.matmul(zc, wb[0:68, WTB0 + 128:WTB0 + 196],
                                         xgb[0:68, j, n0:n1],
                                         start=False, stop=True)

                    # GPSIMD cannot touch PSUM, so the legal engine split is:
                    #   Act:  scaled lo drains (+ gate sigmoids)
                    #   DVE:  hi reconstruction + every other b2 drain
                    #   Pool: b-side hi scale (SBUF-only bf16)
                    nc.scalar.activation(lo_ga[:, jj, :], za_t[:, 0:768],
                                         Copy, scale=crl[:, j:j + 1])
                    nc.scalar.activation(lo_gb[0:68, jj, 0:512], zb1[0:68, :],
                                         Copy, scale=crl[0:68, j:j + 1])
                    nc.scalar.activation(lo_gb[0:68, jj, 512:768],
                                         za_t[0:68, 768:1024], Copy,
                                         scale=crl[0:68, j:j + 1])
                    # hi = crh*x - (crh/crl)*lo: reconstructed from the
                    # drained lo tiles, entirely off PSUM (z has a single
                    # reader).  All-bf16-SBUF muls/adds hit DVE 2x/4x modes;
                    # the b-side x-scale rides the otherwise idle GPSIMD.
                    nc.gpsimd.tensor_scalar_mul(xgb[0:68, j, :],
                                                xgb[0:68, j, :],
                                                crh[0:68, j:j + 1])
                    nc.vector.tensor_scalar_mul(xga[:, j, :],
                                                xga[:, j, :],
                                                crh[:, j:j + 1])
                    ta = scr.tile([128, D], BF16, tag="ta")
                    tb = scr.tile([68, D], BF16, tag="tb")
                    nc.vector.tensor_scalar_mul(ta[:], lo_ga[:, jj, :],
                                                rneg[:, j:j + 1])
                    nc.vector.tensor_add(ho_ga[:, jj, :], xga[:, j, :],
                                         ta[:])
                    nc.vector.tensor_scalar_mul(tb[0:68, :],
                                                lo_gb[0:68, jj, :],
                                                rneg[0:68, j:j + 1])
                    nc.vector.tensor_add(ho_gb[0:68, jj, :], xgb[0:68, j, :],
                                         tb[0:68, :])

                    # ship each completed half immediately (stores from SP:
                    # all loads were pre-issued, so the in-order queue never
                    # parks a blocked store ahead of a ready load)
                    if jj == 1 or j == gn - 1:
                        k0 = j - jj
                        c0, c1 = b0 + k0, b0 + j + 1
                        kn = j + 1 - k0
                        nc.sync.dma_start(
                            out=lo_h[c0:c1, 1:129, :].rearrange(
                                "b t d -> t b d"),
                            in_=lo_ga[:, 0:kn, :])
                        nc.sync.dma_start(
                            out=lo_h[c0:c1, 129:197, :].rearrange(
                                "b t d -> t b d"),
                            in_=lo_gb[:, 0:kn, :])
                        nc.sync.dma_start(
                            out=hi_h[c0:c1, 1:129, :].rearrange(
                                "b t d -> t b d"),
                            in_=ho_ga[:, 0:kn, :])
                        nc.sync.dma_start(
                            out=hi_h[c0:c1, 129:197, :].rearrange(
                                "b t d -> t b d"),
                            in_=ho_gb[:, 0:kn, :])

                # next-next group's gate chain goes ahead of this group's
                # last stores (on PE it runs while Act/DVE/Pool finish here)
                if g + 2 < n_groups:
                    gates[g + 2] = gate_chain(g + 2, *xt[g + 2])

    if not nc.is_finalized():
        nc.finalize()
    return nc


def _make_consts(OP, w1, b1, w2, alpha_low, alpha_high):
    import ml_dtypes
    sig = lambda v: 1.0 / (1.0 + np.exp(-np.float64(v)))
    WT = np.ascontiguousarray(np.asarray(OP, np.float64).T)
    wtblob = np.zeros((128, 400), np.float32)
    wtblob[0:128, 0:196] = WT[0:128]
    wtblob[0:68, 196:392] = WT[128:196]
    wtblob[:, ONES0] = 1.0 / P
    gblob = np.asarray(w1, np.float32).reshape(
        6, 128, 192).transpose(1, 0, 2).reshape(128, 1152)
    w2blob = np.zeros((96, 4), np.float32)
    w2blob[:, 0:2] = np.asarray(w2, np.float32)[0:96]
    w2blob[:, 2:4] = np.asarray(w2, np.float32)[96:192]
    rowblob = np.zeros((1, 592), np.float32)
    rowblob[0, RB_B1:RB_B1 + 192] = np.asarray(b1, np.float32)
    rowblob[0, RB_ONES:RB_ONES + 16] = 1.0
    rowblob[0, RB_ALR:RB_ALR + 128] = sig(alpha_low)
    rowblob[0, RB_AHR:RB_AHR + 128] = sig(alpha_high)
    rowblob[0, RB_CNEG:RB_CNEG + 128] = -sig(alpha_high) / sig(alpha_low)
    return {"wtblob": wtblob.astype(ml_dtypes.bfloat16),
            "gblob": np.ascontiguousarray(gblob.astype(ml_dtypes.bfloat16)),
            "w2blob": w2blob.astype(ml_dtypes.bfloat16),
            "rowblob": rowblob.astype(ml_dtypes.bfloat16)}


def build_for_sim():
    """Program instance for cost-model simulation (dummy weights)."""
    import ml_dtypes
    consts = {
        "wtblob": np.zeros((128, 400), ml_dtypes.bfloat16),
        "gblob": np.ones((128, 1152), ml_dtypes.bfloat16),
        "w2blob": np.ones((96, 4), ml_dtypes.bfloat16),
        "rowblob": np.ones((1, 592), ml_dtypes.bfloat16),
    }
    return _build_program(consts, 0.0, 0.0)


def kernel(x, low_params, high_params, alpha_low, alpha_high,
           w1, b1, w2, b2, cls_token_idx):
    import ml_dtypes
    assert int(cls_token_idx) == 0
    x = np.asarray(x, dtype=np.float32)
    assert x.shape == (B, N, D)

    lm = _freq_mask_np(low_params, True)
    A = _conv_operator(lm)
    share_Y = np.allclose(np.asarray(low_params, np.float32),
                          np.asarray(high_params, np.float32))
    b2v = np.asarray(b2, np.float64).reshape(2)

    xbf = np.ascontiguousarray(x.astype(ml_dtypes.bfloat16))
    xs = xbf.reshape(NCORES, BS, N, D)
    in_maps = [{"xs": np.ascontiguousarray(xs[c])} for c in range(NCORES)]

    def run_once(OP):
        consts = _make_consts(OP, w1, b1, w2, alpha_low, alpha_high)
        nc = _build_program(consts, float(b2v[0]), float(b2v[1]))
        res = run_bass_kernel_spmd(nc, in_maps, core_ids=list(range(NCORES)))
        lo = np.concatenate([np.asarray(r["lo"]) for r in res.results],
                            axis=0).astype(np.float32)
        hi = np.concatenate([np.asarray(r["hi"]) for r in res.results],
                            axis=0).astype(np.float32)
        if getattr(res, "exec_time_ns", None) is not None:
            print(f"HW exec time: {res.exec_time_ns} ns")
        return lo, hi

    if share_Y:
        return run_once(A)
    # generic case (not hit by the reference inputs): hi needs its own
    # operator; run the validated single-operator program twice
    lo, _ = run_once(A)
    Cm = _conv_operator(_freq_mask_np(high_params, True))
    _, hi = run_once(Cm)
    return lo, hi


# revision 128
# speedup vs baseline: 1.0002x; 1.0002x over previous
"""DCT Frequency Splitter — Trainium2 Bass kernel (v3, bf16 end-to-end).

Math: FFT2 -> mask -> IFFT2 -> real is a linear operator on the 196 patch
tokens (per channel): z = A @ patches with A = Re(Finv diag(m) F) (196x196,
real, built on host from the 4 mask params).  With shared mask params the
high path is high = patches - z, so one matmul feeds both outputs:
lo = s_l * z, hi = s_h * (patches - z).

v3 layout decisions (all driven by the TimelineSim cost model):
- bf16 everywhere off-chip: x is converted to bf16 on the host, outputs are
  stored bf16 and upcast on the host.  Halves DMA traffic (the kernel is
  DMA-bound at ~360 GB/s/core); rel-err budget 2e-2 >> bf16's ~4e-3.
- gate-first: per-image token means are computed straight from the x tiles
  with tiny K-contraction matmuls into a PSUM arena (gT in [d, img] layout,
  no PSUM row drain + transpose shuffle), so the gate scales are ready when
  the main matmuls drain and the lo output leaves PSUM already scaled - one
  Activation pass instead of two.
- hi is reconstructed OFF PSUM from the already-drained lo tile:
  hi = crh*x - (crh/crl)*lo via all-SBUF bf16 muls/adds (DVE 2x/4x modes),
  so PSUM has a single reader (the Act drains) and frees fast.
- engine split per image: Act = scaled lo drains (+ sigmoids), DVE = hi
  reconstruction, Pool/GPSIMD = b-side hi scale (it may not touch PSUM),
  PE = matmuls.
- group-batched load DMAs and per-2-image output tiles/stores: dependency
  tracking is tile-granular, so a store's source tile must not outlive its
  own 2 images, or it waits on the whole group.

Sharding: pure data parallel, batch 128 -> 16 per core across 8 cores.
"""

import os
import numpy as np

import concourse.bass as bass
import concourse.bacc as bacc_mod
import concourse.mybir as mybir
import concourse.tile as tile
from concourse.bass_utils import run_bass_kernel_spmd
from concourse.tile_rust import add_dep_helper

H, W = 14, 14
B, N, D = 128, 197, 768
P = 196  # patch tokens
NCORES = 8
BS = B // NCORES  # batches per core

GRP = int(os.environ.get("KRN_GROUP", "4"))        # images per group
BUFO = int(os.environ.get("KRN_BUFO", "5"))        # output half-tile buffers
F32 = mybir.dt.float32
BF16 = mybir.dt.bfloat16

# x tiles hold tokens 1..196 only (CLS skipped at load): xga = patches
# 0..127 on partitions 0..127, xgb = patches 128..195 on partitions 0..67,
# so matmul operands/outputs and the elementwise hi ops all share base
# partition 0 (the PE requires operand base partition in {0, 32, 64}).
# wtblob column layout (bf16, [128, 400])
WTA0 = 0      # A^T rows 0:128 (K = patches 0..127), cols 0:196
WTB0 = 196    # A^T rows 128:196 (K = patches 128..195), cols 196:392
ONES0 = 392   # [128, 1] column of 1/196
# gblob column layout (bf16, [128, 1748])
GW1, GB1 = 0, 1152
GW20, GW21 = 1344, 1346
GONES = 1348
GALR, GAHR = 1364, 1492
GCNEG = 1620  # -sig(alpha_high)/sig(alpha_low) row for the hi-ratio


def _freq_mask_np(params, low):
    ch, cw, radius, sharp = [np.float64(v) for v in np.asarray(params)]
    y = np.arange(H, dtype=np.float64)
    x = np.arange(W, dtype=np.float64)
    d2 = (y[:, None] - ch) ** 2 + (x[None, :] - cw) ** 2
    dist = np.sqrt(d2 + 1e-12)
    s = np.clip(sharp, 0.5, 10.0)
    r = np.clip(radius, 1.0, min(H, W) / 2.0)
    m = np.exp(-((dist / r) ** s))
    return m if low else 1.0 - m


def _conv_operator(mask):
    """Real 196x196 operator equivalent to ifft2(fft2(img)*mask).real."""
    F_H = np.exp(-2j * np.pi * np.outer(np.arange(H), np.arange(H)) / H)
    F_W = np.exp(-2j * np.pi * np.outer(np.arange(W), np.arange(W)) / W)
    Fi_H = np.conj(F_H) / H
    Fi_W = np.conj(F_W) / W
    op = np.kron(Fi_H, Fi_W) @ np.diag(mask.ravel()) @ np.kron(F_H, F_W)
    return np.real(op)


def _build_program(consts, b2lo, b2hi):
    nc = bacc_mod.Bacc(None)

    xs_h = nc.dram_tensor("xs", [BS, N, D], BF16, kind="ExternalInput")
    lo_h = nc.dram_tensor("lo", [BS, N, D], BF16, kind="ExternalOutput")
    hi_h = nc.dram_tensor("hi", [BS, N, D], BF16, kind="ExternalOutput")

    ch = {k: nc.inline_tensor(v, name=f"c_{k}") for k, v in consts.items()}

    Copy = mybir.ActivationFunctionType.Copy
    Sig = mybir.ActivationFunctionType.Sigmoid

    # two small leading groups ramp the store stream earliest (measured
    # best); KRN_GPAT overrides for experiments
    if os.environ.get("KRN_GPAT"):
        sizes = [int(v) for v in os.environ["KRN_GPAT"].split(",")]
        assert sum(sizes) == BS
    elif BS == 16 and GRP == 4:
        sizes = [2, 2, 4, 4, 4]
    else:
        sizes = [min(GRP, BS - s) for s in range(0, BS, GRP)]
    groups, s = [], 0
    for sz in sizes:
        groups.append(list(range(s, s + sz)))
        s += sz
    n_groups = len(groups)

    with tile.TileContext(nc) as tc:
        with (
            tc.tile_pool(name="consts", bufs=1) as cp,
            tc.tile_pool(name="xp", bufs=n_groups) as xp,
            tc.tile_pool(name="outp", bufs=BUFO) as outp,
            tc.tile_pool(name="scr", bufs=4) as scr,
            tc.tile_pool(name="gp", bufs=2) as gp,
            tc.tile_pool(name="pm", bufs=2, space="PSUM") as pm,
            tc.tile_pool(name="par", bufs=2, space="PSUM") as par,
        ):
            def cload(key, dtype):
                arr = consts[key]
                t = cp.tile(list(arr.shape), dtype, tag=key)
                nc.sync.dma_start(out=t[:], in_=ch[key][...])
                return t

            wb = cload("wtblob", BF16)      # [128, 400]
            onescol = wb[:, ONES0:ONES0 + 1]

            def load_group(g):
                bs = groups[g]
                b0, gn = bs[0], len(bs)
                xga = xp.tile([128, GRP, D], BF16, tag="xga")
                xgb = xp.tile([68, GRP, D], BF16, tag="xgb")
                nc.sync.dma_start(
                    out=xga[:, 0:gn, :],
                    in_=xs_h[b0:b0 + gn, 1:129, :].rearrange("b t d -> t b d"))
                nc.sync.dma_start(
                    out=xgb[:, 0:gn, :],
                    in_=xs_h[b0:b0 + gn, 129:197, :].rearrange("b t d -> t b d"))
                return xga, xgb

            # group 0's loads go before the remaining consts so its gate
            # chain starts as early as possible; all loads precede all
            # stores so SP's in-order sequencer never parks a semaphore-
            # blocked store ahead of a ready load
            xt = {0: load_group(0)}

            gb = cload("gblob", BF16)       # [128, 1748]
            w1c = gb[:, GW1:GW1 + 1152].rearrange("p (a h b) -> p a h b",
                                                  a=6, h=2)
            w2c0 = gb[0:96, GW20:GW20 + 2]
            w2c1 = gb[0:96, GW21 + 0:GW21 + 2]
            ones1 = gb[0:1, GONES:GONES + 16]
            alr = gb[0:1, GALR:GALR + 128]
            ahr = gb[0:1, GAHR:GAHR + 128]
            cneg = gb[0:1, GCNEG:GCNEG + 128]

            # dummy activation so the act-func table load (~1.3us) runs at
            # t~=1us instead of stalling the first group's gate sigmoid;
            # reads the earliest const so it never blocks Act's queue
            warm = gp.tile([1, 16], F32, tag="warm")
            nc.scalar.activation(warm[:], wb[0:1, 0:16], Sig)

            for g in range(1, n_groups):
                xt[g] = load_group(g)

            # CLS passthrough for all batches (DRAM -> DRAM), issued from
            # the near-free GPSIMD DMA queue so SP only handles bulk I/O;
            # emitted after the loads so its transfers never wedge between
            # the weight blob and the first x tile on the DMA FIFO
            nc.gpsimd.dma_start(out=lo_h[:, 0:1, :], in_=xs_h[:, 0:1, :])
            nc.gpsimd.dma_start(out=hi_h[:, 0:1, :], in_=xs_h[:, 0:1, :])

            def gate_chain(g, xga, xgb):
                bs = groups[g]
                gn = len(bs)
                # gate for this group, ahead of the main matmuls.
                # gT[d, j] = mean over patch tokens of x (tiny K-contraction
                # matmuls straight into the psum arena, [d, img] layout).
                arena = par.tile([128, 384], F32, tag="arena")
                for j in range(gn):
                    for c in range(6):
                        col = c * 16 + j
                        nc.tensor.matmul(
                            arena[:, col:col + 1],
                            xga[:, j, c * 128:(c + 1) * 128],
                            onescol[:],
                            start=True, stop=False)
                        nc.tensor.matmul(
                            arena[:, col:col + 1],
                            xgb[0:68, j, c * 128:(c + 1) * 128],
                            onescol[0:68],
                            start=False, stop=True)
                gTt = gp.tile([128, 6, 16], BF16, tag="gTt")
                nc.vector.tensor_copy(
                    gTt[:].rearrange("p a b -> p (a b)"), arena[:, 0:96])

                # hidden layer directly in transposed [feature, img] layout
                # (w1 chunks stationary): no transposes, tiny moving dims
                gate_pe = None
                for h in range(2):
                    hps = arena[0:96, 96 + 16 * h:96 + 16 * h + 16]
                    for c in range(6):
                        nc.tensor.matmul(hps[:, 0:gn], w1c[:, c, h, :],
                                         gTt[:, c, 0:gn],
                                         start=(c == 0), stop=False)
                    gate_pe = nc.tensor.matmul(
                        hps[:, 0:gn], gb[0:1, GB1 + 96 * h:GB1 + 96 * h + 96],
                        ones1[0:1, 0:gn], start=False, stop=True)
                hTt = gp.tile([96, 2, 16], BF16, tag="hTt")
                for h in range(2):
                    hps = arena[0:96, 96 + 16 * h:96 + 16 * h + 16]
                    nc.vector.tensor_relu(hTt[:, h, 0:gn], hps[:, 0:gn])

                crows = []
                for col, b2f in ((0, b2lo), (1, b2hi)):
                    g_ps = arena[0:1, 128 + 16 * col:144 + 16 * col]
                    nc.tensor.matmul(g_ps[:, 0:gn], w2c0[:, col:col + 1],
                                     hTt[:, 0, 0:gn], start=True, stop=False)
                    nc.tensor.matmul(g_ps[:, 0:gn], w2c1[:, col:col + 1],
                                     hTt[:, 1, 0:gn], start=False, stop=True)
                    cr = gp.tile([1, 16], BF16, tag=f"crow{col}")
                    nc.scalar.activation(cr[:, 0:gn], g_ps[:, 0:gn], Sig,
                                         bias=b2f)
                    crows.append(cr)
                # per-image hi/lo gate ratio (the hi path is reconstructed
                # from the already-scaled lo tile: hi = crh*x - r*lo with
                # r = crh/crl; the -alpha ratio constant lives in cneg)
                rcp = gp.tile([1, 16], F32, tag="rcp")
                nc.vector.reciprocal(rcp[:, 0:gn], crows[0][0:1, 0:gn])
                rrow = gp.tile([1, 16], BF16, tag="rrow")
                nc.vector.tensor_mul(rrow[:, 0:gn], rcp[:, 0:gn],
                                     crows[1][0:1, 0:gn])
                # replicate the gate rows across partitions; the alpha
                # sigmoid is folded into the alr/ahr/cneg weight rows
                for k, (wrow, mov) in enumerate(
                        ((alr, crows[0][0:1, 0:gn]),
                         (ahr, crows[1][0:1, 0:gn]),
                         (cneg, rrow[0:1, 0:gn]))):
                    nc.tensor.matmul(arena[:, 160 + 16 * k:160 + 16 * k + gn],
                                     wrow[0:1, :], mov, start=True, stop=True)
                crlh = gp.tile([128, 48], F32, tag="crlh")
                nc.vector.tensor_copy(crlh[:], arena[:, 160:208])
                return crlh, gate_pe

            # gates run two groups ahead of their bodies so their small
            # DVE/Act steps never queue behind a full body's engine work
            gates = {g: gate_chain(g, *xt[g]) for g in range(min(2, n_groups))}

            for g, bs in enumerate(groups):
                gn = len(bs)
                b0 = bs[0]
                xga, xgb = xt[g]
                crlh, gate_pe = gates.pop(g)
                crl = crlh[:, 0:16]
                crh = crlh[:, 16:32]
                rneg = crlh[:, 32:48]

                # ---- main matmuls + scaled drains per image
                for j in range(gn):
                    # per-2-image output tiles: a store only waits on its
                    # own half's drains/adds (tile-granular dependency
                    # tracking would otherwise park it behind the whole
                    # group), and dedicated hi tiles release the x tiles to
                    # the pool at the final add rather than at the store
                    jj = j % 2
                    if jj == 0:
                        lo_ga = outp.tile([128, 2, D], BF16, tag="lo_ga")
                        lo_gb = outp.tile([68, 2, D], BF16, tag="lo_gb")
                        ho_ga = outp.tile([128, 2, D], BF16, tag="ho_ga")
                        ho_gb = outp.tile([68, 2, D], BF16, tag="ho_gb")
                    # PSUM packing: 3 banks/image — za_t[128,1024] holds
                    # za(0:768) + zb's tail chunk (768:1024), zb1 the rest.
                    # The a-side then drains in ONE 768-col Act pass; the
                    # two tiles free independently (a-side earlier), which
                    # beats a fully-merged 3-bank tile.
                    za_t = pm.tile([128, 1024], F32, tag="za_t")
                    zb1 = pm.tile([68, 512], F32, tag="zb1")
                    za_ch = [(0, 512, za_t[:, 0:512]),
                             (512, 768, za_t[:, 512:768])]
                    zb_ch = [(0, 512, zb1[0:68, :]),
                             (512, 768, za_t[0:68, 768:1024])]
                    for (n0, n1, zc) in za_ch:
                        mm = nc.tensor.matmul(zc, wb[:, 0:128],
                                              xga[:, j, n0:n1],
                                              start=True, stop=False)
                        add_dep_helper(mm.ins, gate_pe.ins,
                                       reason="gate chain schedules first")
                        nc.tensor.matmul(zc, wb[0:68, WTB0:WTB0 + 128],
                                         xgb[0:68, j, n0:n1],
                                         start=False, stop=True)
                    for (n0, n1, zc) in zb_ch:
                        mm = nc.tensor.matmul(zc, wb[:, 128:196],
                                              xga[:, j, n0:n1],
                                              start=True, stop=False)
                        add_dep_helper(mm.ins, gate_pe.ins,
                                       reason="gate chain schedules first")
                        nc.tensor.matmul(zc, wb[0:68, WTB0 + 128:WTB0 + 196],
                                         xgb[0:68, j, n0:n1],
                                         start=False, stop=True)

                    # GPSIMD cannot touch PSUM, so the legal engine split is:
                    #   Act:  scaled lo drains (+ gate sigmoids)
                    #   DVE:  hi reconstruction + every other b2 drain
                    #   Pool: b-side hi scale (SBUF-only bf16)
                    nc.scalar.activation(lo_ga[:, jj, :], za_t[:, 0:768],
                                         Copy, scale=crl[:, j:j + 1])
                    nc.scalar.activation(lo_gb[0:68, jj, 0:512], zb1[0:68, :],
                                         Copy, scale=crl[0:68, j:j + 1])
                    nc.scalar.activation(lo_gb[0:68, jj, 512:768],
                                         za_t[0:68, 768:1024], Copy,
                                         scale=crl[0:68, j:j + 1])
                    # hi = crh*x - (crh/crl)*lo: reconstructed from the
                    # drained lo tiles, entirely off PSUM (z has a single
                    # reader).  All-bf16-SBUF muls/adds hit DVE 2x/4x modes;
                    # the b-side x-scale rides the otherwise idle GPSIMD.
                    nc.gpsimd.tensor_scalar_mul(xgb[0:68, j, :],
                                                xgb[0:68, j, :],
                                                crh[0:68, j:j + 1])
                    nc.vector.tensor_scalar_mul(xga[:, j, :],
                                                xga[:, j, :],
                                                crh[:, j:j + 1])
                    ta = scr.tile([128, D], BF16, tag="ta")
                    tb = scr.tile([68, D], BF16, tag="tb")
                    nc.vector.tensor_scalar_mul(ta[:], lo_ga[:, jj, :],
                                                rneg[:, j:j + 1])
                    nc.vector.tensor_add(ho_ga[:, jj, :], xga[:, j, :],
                                         ta[:])
                    nc.vector.tensor_scalar_mul(tb[0:68, :],
                                                lo_gb[0:68, jj, :],
                                                rneg[0:68, j:j + 1])
                    nc.vector.tensor_add(ho_gb[0:68, jj, :], xgb[0:68, j, :],
                                         tb[0:68, :])

                    # ship each completed half immediately (stores from SP:
                    # all loads were pre-issued, so the in-order queue never
                    # parks a blocked store ahead of a ready load)
                    if jj == 1 or j == gn - 1:
                        k0 = j - jj
                        c0, c1 = b0 + k0, b0 + j + 1
                        kn = j + 1 - k0
                        nc.sync.dma_start(
                            out=lo_h[c0:c1, 1:129, :].rearrange(
                                "b t d -> t b d"),
                            in_=lo_ga[:, 0:kn, :])
                        nc.sync.dma_start(
                            out=lo_h[c0:c1, 129:197, :].rearrange(
                                "b t d -> t b d"),
                            in_=lo_gb[:, 0:kn, :])
                        nc.sync.dma_start(
                            out=hi_h[c0:c1, 1:129, :].rearrange(
                                "b t d -> t b d"),
                            in_=ho_ga[:, 0:kn, :])
                        nc.sync.dma_start(
                            out=hi_h[c0:c1, 129:197, :].rearrange(
                                "b t d -> t b d"),
                            in_=ho_gb[:, 0:kn, :])

                # next-next group's gate chain goes ahead of this group's
                # last stores (on PE it runs while Act/DVE/Pool finish here)
                if g + 2 < n_groups:
                    gates[g + 2] = gate_chain(g + 2, *xt[g + 2])

    if not nc.is_finalized():
        nc.finalize()
    return nc


def _make_consts(OP, w1, b1, w2, alpha_low, alpha_high):
    import ml_dtypes
    sig = lambda v: 1.0 / (1.0 + np.exp(-np.float64(v)))
    WT = np.ascontiguousarray(np.asarray(OP, np.float64).T)
    wtblob = np.zeros((128, 400), np.float32)
    wtblob[0:128, 0:196] = WT[0:128]
    wtblob[0:68, 196:392] = WT[128:196]
    wtblob[:, ONES0] = 1.0 / P
    gblob = np.zeros((128, 1748), np.float32)
    gblob[:, GW1:GW1 + 1152] = np.asarray(w1, np.float32).reshape(
        6, 128, 192).transpose(1, 0, 2).reshape(128, 1152)
    gblob[0, GB1:GB1 + 192] = np.asarray(b1, np.float32)
    gblob[0:96, GW20:GW20 + 2] = np.asarray(w2, np.float32)[0:96]
    gblob[0:96, GW21:GW21 + 2] = np.asarray(w2, np.float32)[96:192]
    gblob[0, GONES:GONES + 16] = 1.0
    gblob[0, GALR:GALR + 128] = sig(alpha_low)
    gblob[0, GAHR:GAHR + 128] = sig(alpha_high)
    gblob[0, GCNEG:GCNEG + 128] = -sig(alpha_high) / sig(alpha_low)
    return {"wtblob": wtblob.astype(ml_dtypes.bfloat16),
            "gblob": gblob.astype(ml_dtypes.bfloat16)}


def build_for_sim():
    """Program instance for cost-model simulation (dummy weights)."""
    import ml_dtypes
    consts = {
        "wtblob": np.zeros((128, 400), ml_dtypes.bfloat16),
        "gblob": np.ones((128, 1748), ml_dtypes.bfloat16),
    }
    return _build_program(consts, 0.0, 0.0)


def kernel(x, low_params, high_params, alpha_low, alpha_high,
           w1, b1, w2, b2, cls_token_idx):
    import ml_dtypes
    assert int(cls_token_idx) == 0
    x = np.asarray(x, dtype=np.float32)
    assert x.shape == (B, N, D)

    lm = _freq_mask_np(low_params, True)
    A = _conv_operator(lm)
    share_Y = np.allclose(np.asarray(low_params, np.float32),
                          np.asarray(high_params, np.float32))
    b2v = np.asarray(b2, np.float64).reshape(2)

    xbf = np.ascontiguousarray(x.astype(ml_dtypes.bfloat16))
    xs = xbf.reshape(NCORES, BS, N, D)
    in_maps = [{"xs": np.ascontiguousarray(xs[c])} for c in range(NCORES)]

    def run_once(OP):
        consts = _make_consts(OP, w1, b1, w2, alpha_low, alpha_high)
        nc = _build_program(consts, float(b2v[0]), float(b2v[1]))
        res = run_bass_kernel_spmd(nc, in_maps, core_ids=list(range(NCORES)))
        lo = np.concatenate([np.asarray(r["lo"]) for r in res.results],
                            axis=0).astype(np.float32)
        hi = np.concatenate([np.asarray(r["hi"]) for r in res.results],
                            axis=0).astype(np.float32)
        if getattr(res, "exec_time_ns", None) is not None:
            print(f"HW exec time: {res.exec_time_ns} ns")
        return lo, hi

    if share_Y:
        return run_once(A)
    # generic case (not hit by the reference inputs): hi needs its own
    # operator; run the validated single-operator program twice
    lo, _ = run_once(A)
    Cm = _conv_operator(_freq_mask_np(high_params, True))
    _, hi = run_once(Cm)
    return lo, hi
